# revision 1
# baseline (speedup 1.0000x reference)
"""Neural ODE (explicit Euler, 20 steps) Trainium2 Bass kernel.

z_{s+1} = z_s + h * (tanh(z_s @ W1 + b1) @ W2 + b2),  z0: [8192, 512] f32.

Strategy: pure data parallel over 8 NeuronCores (1024 batch rows each).
On each core the state is kept feature-major (zT: [512 features, 1024 batch])
resident in SBUF for all 20 steps; weights are replicated and resident. The
host supplies z pre-transposed (plus a pre-rounded fp16 copy) and receives
the result feature-major, so the device spends no cycles on layout changes;
sharding/unsharding and layout prep are host-side numpy.

Matmuls run with fp16 inputs + fp32 PSUM accumulation (~2e-4 final rel err);
the fp32 master copy of z is updated each step from the fp32 PSUM result, so
fp16 rounding does not accumulate in the state. fp16 streams 1 col/cycle on
the PE (fp32 is 4x slower) and its weight loads use FWL, hiding LDWEIGHTS
under the previous matmul's streaming (216 ns / 512-col matmul measured;
64 matmuls per step per core).

Bias folding: z_s = u_s + s*h*b2, where u_s carries only the matmul updates.
tanh input bias becomes b1 + s*(W1^T (h*b2)) (host-precomputed per step, free
via the ACT bias operand); the final +20*h*b2 correction is accumulated into
the last step's PSUM by a K=1 ones-matmul. With the given inputs b1 = b2 = 0
so all of this is exact regardless.
"""

import numpy as np

P = 128
D = 512
B_FULL = 8192
NCORES = 8
BSH = B_FULL // NCORES  # 1024 batch rows per core
NSTEPS = 20
FT = D // P             # 4 feature tiles
CB = 512                # batch columns per chunk
NCHUNK = BSH // CB      # 2 chunks
NWARM = 14              # data-independent PE prewarm matmuls (HAM clock ramp)

_CACHE = {}


def _build_nc():
    import concourse.bacc as bacc
    import concourse.mybir as mybir
    import concourse.tile as tile
    from concourse.masks import make_identity

    f32 = mybir.dt.float32
    f16 = mybir.dt.float16
    Tanh = mybir.ActivationFunctionType.Tanh

    nc = bacc.Bacc("TRN2", target_bir_lowering=False, debug=False)
    # z transposed on host: [D, BSH] feature-major
    z32_in = nc.dram_tensor("z32", [D, BSH], f32, kind="ExternalInput")
    z16_in = nc.dram_tensor("z16", [D, BSH], f16, kind="ExternalInput")
    w1_in = nc.dram_tensor("w1", [D, D], f16, kind="ExternalInput")
    w2_in = nc.dram_tensor("w2", [D, D], f16, kind="ExternalInput")  # pre-scaled by h
    # biases[p, jt, s] = b1[jt*128+p] + s * (W1^T (h*b2))[jt*128+p]
    b_in = nc.dram_tensor("biases", [P, FT, NSTEPS], f32, kind="ExternalInput")
    # bfin_row[0, j] = NSTEPS * h * b2[j]  (fp16, lhsT of the K=1 bias matmul)
    bf_in = nc.dram_tensor("bfin", [1, D], f16, kind="ExternalInput")
    z_out = nc.dram_tensor("zout", [D, BSH], f32, kind="ExternalOutput")

    z32_t = z32_in.ap().rearrange("(ft p) b -> p ft b", p=P)
    z16_t = z16_in.ap().rearrange("(ft p) b -> p ft b", p=P)
    zout_t = z_out.ap().rearrange("(ft p) b -> p ft b", p=P)

    def cslice(c):
        return slice(c * CB, (c + 1) * CB)

    with tile.TileContext(nc) as tc:
        with (
            tc.tile_pool(name="wpool", bufs=1) as wpool,
            tc.tile_pool(name="zpool", bufs=2) as zpool,
            tc.tile_pool(name="zrpool", bufs=2) as zrpool,
            tc.tile_pool(name="apool", bufs=8) as apool,
            tc.tile_pool(name="ps", bufs=8, space="PSUM") as ps,
        ):
            # ---- identity + PE prewarm (no data deps; ramps the HAM clock
            # to 2.4 GHz while the input DMAs run) ----
            ident = wpool.tile([P, P], f32, tag="id")
            make_identity(nc, ident[:])
            warm_sink = wpool.tile([P, P], f32, tag="warm")
            # preload the tanh ACT table set while DMAs run
            nc.scalar.activation(
                warm_sink[0:1, 0:1], ident[0:1, 0:1],
                mybir.ActivationFunctionType.Tanh,
            )
            for i in range(NWARM):
                wps = ps.tile([P, P], f32, tag="ps", name=f"warm{i}")
                nc.tensor.transpose(wps[:], ident[:], ident[:])
                if i == NWARM - 1:
                    nc.vector.tensor_copy(warm_sink[:], wps[:])

            # ---- input DMAs, one per (chunk, dtype), ordered by first use ----
            z_t = {}   # fp32 master, (c, ft) -> [128, CB]
            zr_t = {}  # fp16 copy for matmul rhs
            zr_init = {}
            zm_init = {}
            for c in range(NCHUNK):
                zr_init[c] = wpool.tile(
                    [P, FT, CB], f16, tag=f"zri{c}", name=f"zri{c}")
                zm_init[c] = wpool.tile(
                    [P, FT, CB], f32, tag=f"zmi{c}", name=f"zmi{c}")
            nc.sync.dma_start(zr_init[0][:], z16_t[:, :, cslice(0)])
            w1r = wpool.tile([P, FT, D], f16, tag="w1r")
            nc.sync.dma_start(w1r[:], w1_in.ap().rearrange("(kt p) j -> p kt j", p=P))
            nc.sync.dma_start(zr_init[1][:], z16_t[:, :, cslice(1)])
            w2r = wpool.tile([P, FT, D], f16, tag="w2r")
            nc.sync.dma_start(w2r[:], w2_in.ap().rearrange("(kt p) j -> p kt j", p=P))
            bias_sb = wpool.tile([P, FT, NSTEPS], f32, tag="bias")
            nc.sync.dma_start(bias_sb[:], b_in.ap())
            nc.sync.dma_start(zm_init[0][:], z32_t[:, :, cslice(0)])
            nc.sync.dma_start(zm_init[1][:], z32_t[:, :, cslice(1)])
            bfin_sb = wpool.tile([1, D], f16, tag="bfin")
            nc.sync.dma_start(bfin_sb[:], bf_in.ap())
            ones_sb = wpool.tile([1, CB], f16, tag="ones")
            nc.vector.memset(ones_sb[:], 1.0)
            for c in range(NCHUNK):
                for ft in range(FT):
                    zr_t[(c, ft)] = zr_init[c][:, ft, :]
                    z_t[(c, ft)] = zm_init[c][:, ft, :]

            # ---- 20 Euler steps ----
            def emit_mm1(s, c):
                    a_t = []
                    for jt in range(FT):
                        ph = ps.tile([P, CB], f32, tag="ps")
                        for kt in range(FT):
                            nc.tensor.matmul(
                                ph[:],
                                w1r[:, kt, jt * P:(jt + 1) * P],
                                zr_t[(c, kt)][:],
                                start=(kt == 0), stop=(kt == FT - 1),
                            )
                        a = apool.tile([P, CB], f16, tag="a")
                        nc.scalar.activation(
                            a[:], ph[:], Tanh, bias=bias_sb[:, jt, s:s + 1], scale=1.0,
                        )
                        a_t.append(a)
                    return a_t

            def emit_mm2(s, c, a_t):
                    last = s == NSTEPS - 1
                    for jt2 in range(FT):
                        py = ps.tile([P, CB], f32, tag="ps")
                        for jt in range(FT):
                            nc.tensor.matmul(
                                py[:],
                                w2r[:, jt, jt2 * P:(jt2 + 1) * P],
                                a_t[jt][:],
                                start=(jt == 0),
                                stop=(jt == FT - 1) and not last,
                            )
                        if last:
                            # += NSTEPS*h*b2 broadcast over batch (K=1 matmul)
                            nc.tensor.matmul(
                                py[:],
                                bfin_sb[:, jt2 * P:(jt2 + 1) * P],
                                ones_sb[:],
                                start=False, stop=True,
                            )
                        z_old = z_t[(c, jt2)]
                        zm = zpool.tile([P, CB], f32, tag=f"z_{c}_{jt2}")
                        nc.vector.tensor_add(zm[:], z_old[:], py[:])
                        z_t[(c, jt2)] = zm
                        if not last:
                            zr = zrpool.tile([P, CB], f16, tag=f"zr_{c}_{jt2}")
                            nc.vector.tensor_add(zr[:], z_old[:], py[:])
                            zr_t[(c, jt2)] = zr
                        else:
                            nc.sync.dma_start(
                                zout_t[:, jt2, cslice(c)], zm[:],
                            )

            a0 = emit_mm1(0, 0)
            a1 = emit_mm1(0, 1)
            emit_mm2(0, 0, a0)
            emit_mm2(0, 1, a1)
            for s in range(1, NSTEPS):
                for c in range(NCHUNK):
                    emit_mm2(s, c, emit_mm1(s, c))

    nc.finalize()
    return nc


def _get_nc():
    if "nc" not in _CACHE:
        _CACHE["nc"] = _build_nc()
    return _CACHE["nc"]


def _prepare_inputs(z0, t, W1, b1, W2, b2):
    z0 = np.asarray(z0, dtype=np.float32)
    t = np.asarray(t, dtype=np.float32)
    W1 = np.asarray(W1, dtype=np.float32)
    b1 = np.asarray(b1, dtype=np.float64)
    W2 = np.asarray(W2, dtype=np.float32)
    b2 = np.asarray(b2, dtype=np.float64)

    zT = np.ascontiguousarray(z0.T)          # [D, B_FULL] f32
    zT16 = zT.astype(np.float16)             # [D, B_FULL] f16

    h = (float(t[1]) - float(t[0])) / NSTEPS
    W1_16 = np.ascontiguousarray(W1.astype(np.float16))
    W2h_16 = np.ascontiguousarray(
        (W2.astype(np.float64) * h).astype(np.float32).astype(np.float16)
    )
    b2h = b2 * h
    wtb = W1.astype(np.float64).T @ b2h  # [D]
    biases = np.stack(
        [b1 + s * wtb for s in range(NSTEPS)], axis=0
    ).astype(np.float32)  # [NSTEPS, D]
    biases_tiled = np.ascontiguousarray(
        biases.reshape(NSTEPS, FT, P).transpose(2, 1, 0)
    )  # [P, FT, NSTEPS]
    bfin_row = np.ascontiguousarray(
        (NSTEPS * b2h).astype(np.float32).astype(np.float16).reshape(1, D)
    )

    in_maps = []
    for i in range(NCORES):
        in_maps.append({
            "z32": np.ascontiguousarray(zT[:, i * BSH:(i + 1) * BSH]),
            "z16": np.ascontiguousarray(zT16[:, i * BSH:(i + 1) * BSH]),
            "w1": W1_16,
            "w2": W2h_16,
            "biases": biases_tiled,
            "bfin": bfin_row,
        })
    return in_maps


def _run(in_maps, trace=False):
    from concourse import bass_utils

    nc = _get_nc()
    res = bass_utils.run_bass_kernel_spmd(
        nc, in_maps, core_ids=list(range(NCORES)), trace=trace,
    )
    return res


def kernel(z0, t, W1, b1, W2, b2):
    in_maps = _prepare_inputs(z0, t, W1, b1, W2, b2)
    res = _run(in_maps)
    outT = np.concatenate([r["zout"] for r in res.results], axis=1)  # [D, B]
    return np.ascontiguousarray(outT.T).astype(np.float32)

